# revision 33
# baseline (speedup 1.0000x reference)
"""Causal multi-head attention block (QKV proj + causal softmax attention + out proj)
for Trainium2, sharded over 8 NeuronCores: data-parallel over batch (2), tensor-
parallel over heads (16 heads -> 4 per core).

Shapes (hardcoded): B=2, T=2048, C=1024, H=16, Dh=64.
Each core computes a partial output projection [T, C] for its 4 heads; the host
sums the 4 partials per batch and adds the fc bias.

Structure: QKV/FC matmul units are interleaved between attention (score->exp->AV)
pairs so the tensor engine never head-of-line blocks on the scalar engine's exp.
Softmax denominators come from a ones-column in V; the reciprocal is broadcast
across partitions with a K=1 ones-matmul on the PE (no DRAM bounce).
"""

import os

import numpy as np

import concourse.bass as bass
import concourse.tile as tile
from concourse import bacc, mybir
from concourse.bass_utils import run_bass_kernel_spmd

F32 = mybir.dt.float32
BF16 = mybir.dt.bfloat16

B = 2
T = 2048
C = 1024
H_PER_CORE = 4  # local heads per core
DH = 64
O_CORE = H_PER_CORE * DH  # 256 output channels per core (per q/k/v)

TCH = 512  # t-chunk size (free dim of most matmuls)
N_CHUNKS = T // TCH  # 4
KT = T // 128  # 16 k-tiles of 128

N_WARM = 64  # PE warmup matmuls issued during the DMA lead-in (keeps HAM warm)

_BUILD_CACHE = {}
LAST_RESULT = None


def build(t=T):
    n_chunks = t // TCH
    nc = bacc.Bacc("TRN2", target_bir_lowering=False)

    xT = nc.declare_dram_parameter("xT", [C, t], BF16, isOutput=False)
    wqkvT = nc.declare_dram_parameter("wqkvT", [C, 3 * O_CORE], BF16, isOutput=False)
    bqk = nc.declare_dram_parameter("bqk", [128, 4], F32, isOutput=False)
    bv_rep = nc.declare_dram_parameter("bv_rep", [128, O_CORE], F32, isOutput=False)
    wfcT = nc.declare_dram_parameter("wfcT", [O_CORE, C], BF16, isOutput=False)
    mask = nc.declare_dram_parameter("mask", [128, 128], BF16, isOutput=False)
    y = nc.declare_dram_parameter("y", [t, C], BF16, isOutput=True)

    with (
        tile.TileContext(nc) as tc,
        tc.tile_pool(name="singles", bufs=1) as singles,
        tc.tile_pool(name="xpool", bufs=4) as xpool,
        tc.tile_pool(name="wtpool", bufs=6) as wtpool,
        tc.tile_pool(name="attnpool", bufs=4) as attnpool,
        tc.tile_pool(name="opool", bufs=4) as opool,
        tc.tile_pool(name="rpool", bufs=4) as rpool,
        tc.tile_pool(name="dpool", bufs=8, space="DRAM") as dpool,
        tc.tile_pool(name="mmps", bufs=2, space="PSUM") as mmps,
        tc.tile_pool(name="sps", bufs=2, space="PSUM") as sps,
        tc.tile_pool(name="avps", bufs=2, space="PSUM") as avps,
    ):
        # ---- t=0: memsets + PE warmup spin through the DMA lead-in ----
        warm = singles.tile([128, TCH], BF16)
        nc.vector.memset(warm[:], 0.25)
        v_sb = singles.tile([128, t // 128, H_PER_CORE, DH + 1], BF16)
        nc.vector.memset(v_sb[:, :, :, DH : DH + 1], 1.0)  # ones col -> softmax denom

        wps = mmps.tile([128, TCH], F32, tag="mm", name="warm")
        for _ in range(N_WARM):
            nc.tensor.matmul(
                wps[:, 0:128], warm[:, 0:128], warm[:, 0:128], start=True, stop=True,
                skip_group_check=True,
            )

        # ---- startup DMA burst, split across 3 queues ----
        xT_r = xT.rearrange("(co ci) t -> ci co t", ci=128)
        wqkvT_r = wqkvT.rearrange("(co ci) o -> ci co o", ci=128)
        wq_sb = singles.tile([128, 8, 3 * O_CORE], BF16)  # [ci, co, o] = wqkvT
        xt0 = xpool.tile([128, 8, TCH], BF16, tag="xt", name="xt0")
        for co in range(8):
            nc.sync.dma_start(xt0[:, co, :], xT_r[:, co, :TCH])
            eng = nc.gpsimd if co < 4 else nc.scalar
            eng.dma_start(wq_sb[:, co, :], wqkvT_r[:, co, :])
        bqk_sb = singles.tile([128, 4], F32)
        nc.sync.dma_start(bqk_sb[:], bqk[:])
        mask_sb = singles.tile([128, 128], BF16)
        nc.sync.dma_start(mask_sb[:], mask[:])
        bv_sb = singles.tile([128, H_PER_CORE, DH], F32)
        nc.gpsimd.dma_start(bv_sb[:], bv_rep.rearrange("p (h d) -> p h d", h=H_PER_CORE))
        wfc_sb = singles.tile([128, 2, C], BF16)  # [p, ks, n]
        nc.scalar.dma_start(wfc_sb[:], wfcT.rearrange("(ks p) n -> p ks n", p=128))

        qT_sb = singles.tile([128, 2, t], BF16)  # [dh + 64*(h%2), h//2, t]
        kT_sb = singles.tile([128, 2, t], BF16)

        def load_xt(tcix):
            ts0 = tcix * TCH
            xt = xpool.tile([128, 8, TCH], BF16, tag="xt", name=f"xt{tcix}")
            for co in range(8):
                eng = nc.sync if co % 2 == 0 else nc.gpsimd
                eng.dma_start(xt[:, co, :], xT_r[:, co, ts0 : ts0 + TCH])
            return xt

        # prefetch all remaining x chunks up front (xpool holds all 4)
        xts = [xt0] + [load_xt(i) for i in range(1, n_chunks)]

        def qkv_units(tcix, xt):
            ts0 = tcix * TCH

            def qk_group(i):
                ps = mmps.tile([128, TCH], F32, tag="mm", name="qkps")
                for co in range(8):
                    nc.tensor.matmul(
                        ps[:],
                        wq_sb[:, co, i * 128 : (i + 1) * 128],
                        xt[:, co, :],
                        start=(co == 0),
                        stop=(co == 7),
                    )
                dst = qT_sb if i < 2 else kT_sb
                nc.vector.tensor_scalar_add(
                    dst[:, i % 2, ts0 : ts0 + TCH], ps[:], bqk_sb[:, i : i + 1]
                )

            def v_group(j):
                ps = mmps.tile([128, TCH], F32, tag="mm", name="vps")
                for half in range(2):
                    tt = j * 2 + half
                    for co in range(8):
                        nc.tensor.matmul(
                            ps[:, half * 256 : (half + 1) * 256],
                            xt[:, co, tt * 128 : (tt + 1) * 128],
                            wq_sb[:, co, 2 * O_CORE : 3 * O_CORE],
                            start=(co == 0),
                            stop=(co == 7),
                        )
                kt0 = tcix * 4 + j * 2
                nc.vector.tensor_add(
                    v_sb[:, kt0 : kt0 + 2, :, 0:DH],
                    ps.rearrange("p (a h d) -> p a h d", a=2, h=H_PER_CORE),
                    bv_sb[:, None, :, :].to_broadcast((128, 2, H_PER_CORE, DH)),
                )

            units = [lambda i=i: qk_group(i) for i in range(4)]
            units += [lambda j=j: v_group(j) for j in range(2)]
            return units

        def fc_units(ts0, attn_t):
            def one(tt, nn):
                ps = mmps.tile([128, TCH], F32, tag="mm", name="fcps")
                for ks in range(2):
                    nc.tensor.matmul(
                        ps[:],
                        attn_t[:, ks, tt * 128 : (tt + 1) * 128],
                        wfc_sb[:, ks, nn * TCH : (nn + 1) * TCH],
                        start=(ks == 0),
                        stop=(ks == 1),
                    )
                ot = opool.tile([128, TCH], BF16, tag="o")
                nc.vector.tensor_copy(ot[:], ps[:])
                nc.sync.dma_start(
                    y[ts0 + tt * 128 : ts0 + (tt + 1) * 128, nn * TCH : (nn + 1) * TCH],
                    ot[:],
                )

            return [lambda tt=tt, nn=nn: one(tt, nn) for tt in range(4) for nn in range(2)]

        def fc_units_end(ts0, attn_t):
            """FC for the final chunk: psum copies alternate Vector/Scalar (both
            idle in the endgame) so the copy chain doesn't pace the tail."""

            def one(tt, nn, idx):
                ps = mmps.tile([128, TCH], F32, tag="mm", name="fcs")
                for ks in range(2):
                    nc.tensor.matmul(
                        ps[:],
                        attn_t[:, ks, tt * 128 : (tt + 1) * 128],
                        wfc_sb[:, ks, nn * TCH : (nn + 1) * TCH],
                        start=(ks == 0),
                        stop=(ks == 1),
                    )
                ot = opool.tile([128, TCH], BF16, tag="o")
                if idx % 2 == 1:
                    nc.scalar.activation(
                        ot[:], ps[:], mybir.ActivationFunctionType.Copy
                    )
                else:
                    nc.vector.tensor_copy(ot[:], ps[:])
                nc.sync.dma_start(
                    y[ts0 + tt * 128 : ts0 + (tt + 1) * 128, nn * TCH : (nn + 1) * TCH],
                    ot[:],
                )

            return [
                lambda tt=tt, nn=nn, idx=idx: one(tt, nn, idx)
                for idx, (tt, nn) in enumerate(
                    (t_, n_) for t_ in range(4) for n_ in range(2)
                )
            ]

        fcq = []
        for tcix in range(n_chunks):
            ts0 = tcix * TCH
            if tcix == 0:
                for u in qkv_units(0, xts[0]):
                    u()
            attn_t = attnpool.tile([128, 2, TCH], BF16, tag="attn")
            # filler units: next chunk's QKV + previous chunk's FC, spread across
            # this chunk's attention pairs to keep the PE fed while ACT does exp.
            # For the last chunk, hp=1's filler is this chunk's own FC ks=0 half
            # (heads 0/1 are final once hp=0's normalize lands).
            if tcix + 1 < n_chunks:
                work = qkv_units(tcix + 1, xts[tcix + 1])
                if fcq:
                    work += fc_units(*fcq.pop(0))
            else:
                work = fc_units(*fcq.pop(0))
            fillers = [work[: (len(work) + 1) // 2], work[(len(work) + 1) // 2 :]]

            kimax = tcix * 4 + 3
            npairs = 2 * tcix + 2
            for hp in range(2):
                heads = (2 * hp, 2 * hp + 1)
                ps_av = {}
                for h in heads:
                    ps_av[h] = avps.tile([128, TCH], F32, tag="av", name=f"av{h}")
                wts = {}
                pend = []

                def do_av(item):
                    h, pj = item
                    wt = wts[(h, pj)]
                    for u in range(2):
                        ki = 2 * pj + u
                        sx = max(0, (ki - 4 * tcix) * 128)
                        nc.tensor.matmul(
                            ps_av[h][0 : DH + 1, sx:TCH],
                            v_sb[:, ki, h, :],
                            wt[:, u, sx:TCH],
                            start=(ki == 0),
                            stop=(ki == kimax),
                            skip_group_check=True,
                        )

                filler = fillers[hp]
                nfill = 0
                for pj in range(npairs):
                    due = len(filler) * (pj + 1) // npairs
                    while nfill < due:
                        filler[nfill]()
                        nfill += 1

                    # deferred AVs first: puts PE work between the two heads'
                    # staging frees so both heads' score MMs queue adjacently
                    # and PE row-group packing engages
                    while len(pend) > 2:
                        do_av(pend.pop(0))

                    # scores: h0/h1 alternating so PE row-group packing (K=64)
                    # can run the two heads concurrently when both are ready
                    st = {}
                    for h in heads:
                        st[h] = sps.tile([128, 2, TCH], F32, tag="s", name=f"st{h}")
                    for u in range(2):
                        ki = 2 * pj + u
                        # partial blocks: only columns >= sx are consumed
                        # downstream (m=1's stale prefix feeds an unread,
                        # bounded exp), so skip streaming the dead columns
                        m = ki - 4 * tcix
                        sx = 128 * m if m >= 2 else 0
                        for h in heads:
                            pb = (h % 2) * 64
                            ho = h // 2
                            nc.tensor.matmul(
                                st[h][:, u, sx:],
                                kT_sb[pb : pb + 64, ho, ki * 128 : (ki + 1) * 128],
                                qT_sb[pb : pb + 64, ho, ts0 + sx : ts0 + TCH],
                                start=True,
                                stop=True,
                                skip_group_check=True,
                            )
                    for h in heads:
                        wt = wtpool.tile([128, 2, TCH], BF16, tag="wt", name=f"wt{h}")
                        m0 = 2 * pj - 4 * tcix  # block offset of the pair's first ki
                        if m0 >= 2:
                            # mostly-masked final pair: exp only the live columns
                            for u in range(2):
                                sx = (m0 + u) * 128
                                nc.scalar.activation(
                                    wt[:, u, sx:TCH],
                                    st[h][:, u, sx:TCH],
                                    mybir.ActivationFunctionType.Exp,
                                    scale=0.125,
                                )
                        else:
                            nc.scalar.activation(
                                wt[:],
                                st[h][:],
                                mybir.ActivationFunctionType.Exp,
                                scale=0.125,
                            )
                        for u in range(2):
                            ki = 2 * pj + u
                            m = ki - 4 * tcix
                            if m >= 0:
                                sx = m * 128
                                nc.vector.tensor_mul(
                                    wt[:, u, sx : sx + 128],
                                    wt[:, u, sx : sx + 128],
                                    mask_sb[:],
                                )
                        wts[(h, pj)] = wt
                        pend.append((h, pj))
                while nfill < len(filler):
                    filler[nfill]()
                    nfill += 1
                while pend:
                    do_av(pend.pop(0))

                if tcix + 1 == n_chunks and hp == 1:
                    # final normalize chains are fully exposed: keep the PE busy
                    # and HAM warm while the DMA round-trips complete
                    wps2 = mmps.tile([128, TCH], F32, tag="mm", name="warm2")
                    for _ in range(50):
                        nc.tensor.matmul(
                            wps2[:], warm[:, 0:128], warm[:],
                            start=True, stop=True, skip_group_check=True,
                        )

                for h in heads:
                    pb = (h % 2) * 64
                    ho = h // 2
                    # alternate DMA queues so the two heads' chains parallelize
                    deng = nc.gpsimd if h % 2 == 0 else nc.sync
                    # free the AV psum quickly: one staging copy of attn + denom
                    stage = rpool.tile([128, TCH], F32, tag="stage")
                    nc.vector.tensor_copy(stage[0 : DH + 1, :], ps_av[h][0 : DH + 1, :])
                    # reciprocal on a [128, 4] reshape (DRAM bounce, so all 128
                    # DVE lanes work), then partition-broadcast read from DRAM
                    d1 = dpool.tile([1, TCH], F32)
                    deng.dma_start(d1[:], stage[DH : DH + 1, :])
                    rp = rpool.tile([128, 4], F32, tag="rp")
                    deng.dma_start(
                        rp[:],
                        bass.AP(tensor=d1.tensor, offset=d1.offset, ap=[[4, 128], [1, 4]]),
                    )
                    rcp = rpool.tile([128, 4], F32, tag="rcp")
                    nc.vector.reciprocal(rcp[:], rp[:])
                    d2 = dpool.tile([1, TCH], F32)
                    deng.dma_start(
                        bass.AP(tensor=d2.tensor, offset=d2.offset, ap=[[4, 128], [1, 4]]),
                        rcp[:],
                    )
                    rep = rpool.tile([128, TCH], F32, tag="rep")
                    deng.dma_start(
                        rep[0:64, :],
                        bass.AP(tensor=d2.tensor, offset=d2.offset, ap=[[0, 64], [1, TCH]]),
                    )
                    nc.vector.tensor_mul(
                        attn_t[pb : pb + 64, ho, :], stage[0:DH, :], rep[0:64, :]
                    )

            if tcix + 1 < n_chunks:
                fcq.append((ts0, attn_t))
            else:
                for u in fc_units_end(ts0, attn_t):
                    u()

    nc.compile()
    return nc


def _prep_core_inputs(x, w_qkv, b_qkv, w_fc, b_fc, core):
    b, g = core // 4, core % 4
    rq = slice(256 * g, 256 * g + 256)
    rk = slice(1024 + 256 * g, 1024 + 256 * g + 256)
    rv = slice(2048 + 256 * g, 2048 + 256 * g + 256)
    wcat = np.concatenate([w_qkv[rq], w_qkv[rk], w_qkv[rv]], axis=0)  # [768, 1024]
    bq, bk, bv = b_qkv[rq], b_qkv[rk], b_qkv[rv]
    import ml_dtypes

    bf16 = ml_dtypes.bfloat16
    return {
        "xT": np.ascontiguousarray(x[b].T).astype(bf16),
        "wqkvT": np.ascontiguousarray(wcat.T).astype(bf16),
        "bqk": np.ascontiguousarray(
            np.stack([bq[0:128], bq[128:256], bk[0:128], bk[128:256]], axis=1)
        ),
        "bv_rep": np.ascontiguousarray(np.broadcast_to(bv, (128, 256))),
        "wfcT": np.ascontiguousarray(w_fc[:, 256 * g : 256 * g + 256].T).astype(bf16),
        "mask": np.triu(np.ones((128, 128), dtype=np.float32)).astype(bf16),
    }


def kernel(x, w_qkv, b_qkv, w_fc, b_fc):
    global LAST_RESULT
    x = np.asarray(x, dtype=np.float32)
    w_qkv = np.asarray(w_qkv, dtype=np.float32)
    b_qkv = np.asarray(b_qkv, dtype=np.float32)
    w_fc = np.asarray(w_fc, dtype=np.float32)
    b_fc = np.asarray(b_fc, dtype=np.float32)

    if "nc" not in _BUILD_CACHE:
        _BUILD_CACHE["nc"] = build()
    nc = _BUILD_CACHE["nc"]

    in_maps = [
        _prep_core_inputs(x, w_qkv, b_qkv, w_fc, b_fc, core) for core in range(8)
    ]
    res = run_bass_kernel_spmd(
        nc,
        in_maps,
        core_ids=list(range(8)),
        trace=bool(os.environ.get("MHA_TRACE")),
    )
    LAST_RESULT = res

    out = np.empty((B, T, C), dtype=np.float32)
    for b in range(B):
        acc = res.results[4 * b]["y"].astype(np.float32)
        for g in range(1, 4):
            acc = acc + res.results[4 * b + g]["y"].astype(np.float32)
        out[b] = acc + b_fc
    return out


# revision 36
# speedup vs baseline: 1.1433x; 1.1433x over previous
"""Causal multi-head attention block (QKV proj + causal softmax attention + out proj)
for Trainium2, sharded over 8 NeuronCores: data-parallel over batch (2), tensor-
parallel over heads (16 heads -> 4 per core).

Shapes (hardcoded): B=2, T=2048, C=1024, H=16, Dh=64.
Each core computes a partial output projection [T, C] for its 4 heads; the host
sums the 4 partials per batch and adds the fc bias.

Structure: QKV/FC matmul units are interleaved between attention (score->exp->AV)
pairs so the tensor engine never head-of-line blocks on the scalar engine's exp.
Softmax denominators come from a ones-column in V; the reciprocal is broadcast
across partitions with a K=1 ones-matmul on the PE (no DRAM bounce).
"""

import os

import numpy as np

import concourse.bass as bass
import concourse.tile as tile
from concourse import bacc, mybir
from concourse.bass_utils import run_bass_kernel_spmd

F32 = mybir.dt.float32
BF16 = mybir.dt.bfloat16

B = 2
T = 2048
C = 1024
H_PER_CORE = 4  # local heads per core
DH = 64
O_CORE = H_PER_CORE * DH  # 256 output channels per core (per q/k/v)

TCH = 512  # t-chunk size (free dim of most matmuls)
N_CHUNKS = T // TCH  # 4
KT = T // 128  # 16 k-tiles of 128

N_WARM = 64  # PE warmup matmuls issued during the DMA lead-in (keeps HAM warm)

_BUILD_CACHE = {}
LAST_RESULT = None


def build(t=T):
    n_chunks = t // TCH
    nc = bacc.Bacc("TRN2", target_bir_lowering=False)

    xT = nc.declare_dram_parameter("xT", [C, t], BF16, isOutput=False)
    wqkvT = nc.declare_dram_parameter("wqkvT", [C, 3 * O_CORE], BF16, isOutput=False)
    bqk = nc.declare_dram_parameter("bqk", [128, 4], F32, isOutput=False)
    bv_rep = nc.declare_dram_parameter("bv_rep", [128, O_CORE], F32, isOutput=False)
    wfcT = nc.declare_dram_parameter("wfcT", [O_CORE, C], BF16, isOutput=False)
    mask = nc.declare_dram_parameter("mask", [128, 128], BF16, isOutput=False)
    y = nc.declare_dram_parameter("y", [t, C], BF16, isOutput=True)

    with (
        tile.TileContext(nc) as tc,
        tc.tile_pool(name="singles", bufs=1) as singles,
        tc.tile_pool(name="xpool", bufs=4) as xpool,
        tc.tile_pool(name="wtpool", bufs=6) as wtpool,
        tc.tile_pool(name="attnpool", bufs=4) as attnpool,
        tc.tile_pool(name="opool", bufs=4) as opool,
        tc.tile_pool(name="rpool", bufs=4) as rpool,
        tc.tile_pool(name="dpool", bufs=8, space="DRAM") as dpool,
        tc.tile_pool(name="mmps", bufs=2, space="PSUM") as mmps,
        tc.tile_pool(name="sps", bufs=2, space="PSUM") as sps,
        tc.tile_pool(name="avps", bufs=2, space="PSUM") as avps,
    ):
        # ---- t=0: memsets + PE warmup spin through the DMA lead-in ----
        warm = singles.tile([128, TCH], BF16)
        nc.vector.memset(warm[:], 0.25)
        v_sb = singles.tile([128, t // 128, H_PER_CORE, DH + 1], BF16)
        nc.vector.memset(v_sb[:, :, :, DH : DH + 1], 1.0)  # ones col -> softmax denom

        wps = mmps.tile([128, TCH], F32, tag="mm", name="warm")
        for _ in range(N_WARM):
            nc.tensor.matmul(
                wps[:, 0:128], warm[:, 0:128], warm[:, 0:128], start=True, stop=True,
                skip_group_check=True,
            )

        # ---- startup DMA burst, split across 3 queues ----
        xT_r = xT.rearrange("(co ci) t -> ci co t", ci=128)
        wqkvT_r = wqkvT.rearrange("(co ci) o -> ci co o", ci=128)
        wq_sb = singles.tile([128, 8, 3 * O_CORE], BF16)  # [ci, co, o] = wqkvT
        xt0 = xpool.tile([128, 8, TCH], BF16, tag="xt", name="xt0")
        for co in range(8):
            nc.sync.dma_start(xt0[:, co, :], xT_r[:, co, :TCH])
            eng = nc.gpsimd if co < 4 else nc.scalar
            eng.dma_start(wq_sb[:, co, :], wqkvT_r[:, co, :])
        bqk_sb = singles.tile([128, 4], F32)
        nc.sync.dma_start(bqk_sb[:], bqk[:])
        mask_sb = singles.tile([128, 128], BF16)
        nc.sync.dma_start(mask_sb[:], mask[:])
        bv_sb = singles.tile([128, H_PER_CORE, DH], F32)
        nc.gpsimd.dma_start(bv_sb[:], bv_rep.rearrange("p (h d) -> p h d", h=H_PER_CORE))
        wfc_sb = singles.tile([128, 2, C], BF16)  # [p, ks, n]
        nc.scalar.dma_start(wfc_sb[:], wfcT.rearrange("(ks p) n -> p ks n", p=128))

        qT_sb = singles.tile([128, 2, t], BF16)  # [dh + 64*(h%2), h//2, t]
        kT_sb = singles.tile([128, 2, t], BF16)

        def load_xt(tcix):
            ts0 = tcix * TCH
            xt = xpool.tile([128, 8, TCH], BF16, tag="xt", name=f"xt{tcix}")
            for co in range(8):
                eng = nc.sync if co % 2 == 0 else nc.gpsimd
                eng.dma_start(xt[:, co, :], xT_r[:, co, ts0 : ts0 + TCH])
            return xt

        # prefetch all remaining x chunks up front (xpool holds all 4)
        xts = [xt0] + [load_xt(i) for i in range(1, n_chunks)]

        def qkv_units(tcix, xt, act_bias=False):
            ts0 = tcix * TCH

            def qk_group(i):
                ps = mmps.tile([128, TCH], F32, tag="mm", name="qkps")
                for co in range(8):
                    nc.tensor.matmul(
                        ps[:],
                        wq_sb[:, co, i * 128 : (i + 1) * 128],
                        xt[:, co, :],
                        start=(co == 0),
                        stop=(co == 7),
                    )
                dst = qT_sb if i < 2 else kT_sb
                if act_bias:
                    # chunk 0: ACT is idle during the pipeline fill; DVE is the
                    # scores' critical path there
                    nc.scalar.activation(
                        dst[:, i % 2, ts0 : ts0 + TCH],
                        ps[:],
                        mybir.ActivationFunctionType.Identity,
                        bias=bqk_sb[:, i : i + 1],
                    )
                else:
                    nc.vector.tensor_scalar_add(
                        dst[:, i % 2, ts0 : ts0 + TCH], ps[:], bqk_sb[:, i : i + 1]
                    )

            def v_group(j):
                ps = mmps.tile([128, TCH], F32, tag="mm", name="vps")
                for half in range(2):
                    tt = j * 2 + half
                    for co in range(8):
                        nc.tensor.matmul(
                            ps[:, half * 256 : (half + 1) * 256],
                            xt[:, co, tt * 128 : (tt + 1) * 128],
                            wq_sb[:, co, 2 * O_CORE : 3 * O_CORE],
                            start=(co == 0),
                            stop=(co == 7),
                        )
                kt0 = tcix * 4 + j * 2
                nc.vector.tensor_add(
                    v_sb[:, kt0 : kt0 + 2, :, 0:DH],
                    ps.rearrange("p (a h d) -> p a h d", a=2, h=H_PER_CORE),
                    bv_sb[:, None, :, :].to_broadcast((128, 2, H_PER_CORE, DH)),
                )

            units = [lambda i=i: qk_group(i) for i in range(4)]
            units += [lambda j=j: v_group(j) for j in range(2)]
            return units

        def fc_units(ts0, attn_t):
            def one(tt, nn):
                ps = mmps.tile([128, TCH], F32, tag="mm", name="fcps")
                for ks in range(2):
                    nc.tensor.matmul(
                        ps[:],
                        attn_t[:, ks, tt * 128 : (tt + 1) * 128],
                        wfc_sb[:, ks, nn * TCH : (nn + 1) * TCH],
                        start=(ks == 0),
                        stop=(ks == 1),
                    )
                ot = opool.tile([128, TCH], BF16, tag="o")
                nc.vector.tensor_copy(ot[:], ps[:])
                nc.sync.dma_start(
                    y[ts0 + tt * 128 : ts0 + (tt + 1) * 128, nn * TCH : (nn + 1) * TCH],
                    ot[:],
                )

            return [lambda tt=tt, nn=nn: one(tt, nn) for tt in range(4) for nn in range(2)]

        def fc_units_end(ts0, attn_t):
            """FC for the final chunk: psum copies alternate Vector/Scalar (both
            idle in the endgame) so the copy chain doesn't pace the tail."""

            def one(tt, nn, idx):
                ps = mmps.tile([128, TCH], F32, tag="mm", name="fcs")
                for ks in range(2):
                    nc.tensor.matmul(
                        ps[:],
                        attn_t[:, ks, tt * 128 : (tt + 1) * 128],
                        wfc_sb[:, ks, nn * TCH : (nn + 1) * TCH],
                        start=(ks == 0),
                        stop=(ks == 1),
                    )
                ot = opool.tile([128, TCH], BF16, tag="o")
                if idx % 2 == 1:
                    nc.scalar.activation(
                        ot[:], ps[:], mybir.ActivationFunctionType.Copy
                    )
                else:
                    nc.vector.tensor_copy(ot[:], ps[:])
                nc.sync.dma_start(
                    y[ts0 + tt * 128 : ts0 + (tt + 1) * 128, nn * TCH : (nn + 1) * TCH],
                    ot[:],
                )

            return [
                lambda tt=tt, nn=nn, idx=idx: one(tt, nn, idx)
                for idx, (tt, nn) in enumerate(
                    (t_, n_) for t_ in range(4) for n_ in range(2)
                )
            ]

        fcq = []
        for tcix in range(n_chunks):
            ts0 = tcix * TCH
            if tcix == 0:
                for u in qkv_units(0, xts[0], act_bias=True):
                    u()
            attn_t = attnpool.tile([128, 2, TCH], BF16, tag="attn")
            # filler units: next chunk's QKV + previous chunk's FC, spread across
            # this chunk's attention pairs to keep the PE fed while ACT does exp.
            # For the last chunk, hp=1's filler is this chunk's own FC ks=0 half
            # (heads 0/1 are final once hp=0's normalize lands).
            if tcix + 1 < n_chunks:
                work = qkv_units(tcix + 1, xts[tcix + 1])
                if fcq:
                    work += fc_units(*fcq.pop(0))
            else:
                work = fc_units(*fcq.pop(0))
            fillers = [work[: (len(work) + 1) // 2], work[(len(work) + 1) // 2 :]]

            kimax = tcix * 4 + 3
            npairs = 2 * tcix + 2
            for hp in range(2):
                heads = (2 * hp, 2 * hp + 1)
                ps_av = {}
                for h in heads:
                    ps_av[h] = avps.tile([128, TCH], F32, tag="av", name=f"av{h}")
                wts = {}
                pend = []

                def do_av(item):
                    h, pj = item
                    wt = wts[(h, pj)]
                    for u in range(2):
                        ki = 2 * pj + u
                        sx = max(0, (ki - 4 * tcix) * 128)
                        nc.tensor.matmul(
                            ps_av[h][0 : DH + 1, sx:TCH],
                            v_sb[:, ki, h, :],
                            wt[:, u, sx:TCH],
                            start=(ki == 0),
                            stop=(ki == kimax),
                            skip_group_check=True,
                        )

                filler = fillers[hp]
                nfill = 0
                for pj in range(npairs):
                    due = len(filler) * (pj + 1) // npairs
                    while nfill < due:
                        filler[nfill]()
                        nfill += 1

                    # deferred AVs first: puts PE work between the two heads'
                    # staging frees so both heads' score MMs queue adjacently
                    # and PE row-group packing engages
                    while len(pend) > 2:
                        do_av(pend.pop(0))

                    # scores: h0/h1 alternating so PE row-group packing (K=64)
                    # can run the two heads concurrently when both are ready
                    st = {}
                    for h in heads:
                        st[h] = sps.tile([128, 2, TCH], F32, tag="s", name=f"st{h}")
                    for u in range(2):
                        ki = 2 * pj + u
                        # partial blocks: only columns >= sx are consumed
                        # downstream (m=1's stale prefix feeds an unread,
                        # bounded exp), so skip streaming the dead columns
                        m = ki - 4 * tcix
                        sx = 128 * m if m >= 2 else 0
                        for h in heads:
                            pb = (h % 2) * 64
                            ho = h // 2
                            nc.tensor.matmul(
                                st[h][:, u, sx:],
                                kT_sb[pb : pb + 64, ho, ki * 128 : (ki + 1) * 128],
                                qT_sb[pb : pb + 64, ho, ts0 + sx : ts0 + TCH],
                                start=True,
                                stop=True,
                                skip_group_check=True,
                            )
                    for h in heads:
                        wt = wtpool.tile([128, 2, TCH], BF16, tag="wt", name=f"wt{h}")
                        m0 = 2 * pj - 4 * tcix  # block offset of the pair's first ki
                        if m0 >= 2:
                            # mostly-masked final pair: exp only the live columns
                            for u in range(2):
                                sx = (m0 + u) * 128
                                nc.scalar.activation(
                                    wt[:, u, sx:TCH],
                                    st[h][:, u, sx:TCH],
                                    mybir.ActivationFunctionType.Exp,
                                    scale=0.125,
                                )
                        else:
                            nc.scalar.activation(
                                wt[:],
                                st[h][:],
                                mybir.ActivationFunctionType.Exp,
                                scale=0.125,
                            )
                        for u in range(2):
                            ki = 2 * pj + u
                            m = ki - 4 * tcix
                            if m >= 0:
                                sx = m * 128
                                nc.vector.tensor_mul(
                                    wt[:, u, sx : sx + 128],
                                    wt[:, u, sx : sx + 128],
                                    mask_sb[:],
                                )
                        wts[(h, pj)] = wt
                        pend.append((h, pj))
                while nfill < len(filler):
                    filler[nfill]()
                    nfill += 1
                while pend:
                    do_av(pend.pop(0))

                if tcix + 1 == n_chunks and hp == 1:
                    # final normalize chains are fully exposed: keep the PE busy
                    # and HAM warm while the DMA round-trips complete
                    wps2 = mmps.tile([128, TCH], F32, tag="mm", name="warm2")
                    for _ in range(62):
                        nc.tensor.matmul(
                            wps2[:], warm[:, 0:128], warm[:],
                            start=True, stop=True, skip_group_check=True,
                        )

                for h in heads:
                    pb = (h % 2) * 64
                    ho = h // 2
                    # alternate DMA queues so the two heads' chains parallelize
                    deng = nc.gpsimd if h % 2 == 0 else nc.sync
                    # free the AV psum quickly: one staging copy of attn + denom
                    stage = rpool.tile([128, TCH], F32, tag="stage")
                    nc.vector.tensor_copy(stage[0 : DH + 1, :], ps_av[h][0 : DH + 1, :])
                    # reciprocal on a [128, 4] reshape (DRAM bounce, so all 128
                    # DVE lanes work), then partition-broadcast read from DRAM
                    d1 = dpool.tile([1, TCH], F32)
                    deng.dma_start(d1[:], stage[DH : DH + 1, :])
                    rp = rpool.tile([128, 4], F32, tag="rp")
                    deng.dma_start(
                        rp[:],
                        bass.AP(tensor=d1.tensor, offset=d1.offset, ap=[[4, 128], [1, 4]]),
                    )
                    rcp = rpool.tile([128, 4], F32, tag="rcp")
                    nc.vector.reciprocal(rcp[:], rp[:])
                    d2 = dpool.tile([1, TCH], F32)
                    deng.dma_start(
                        bass.AP(tensor=d2.tensor, offset=d2.offset, ap=[[4, 128], [1, 4]]),
                        rcp[:],
                    )
                    rep = rpool.tile([128, TCH], F32, tag="rep")
                    deng.dma_start(
                        rep[0:64, :],
                        bass.AP(tensor=d2.tensor, offset=d2.offset, ap=[[0, 64], [1, TCH]]),
                    )
                    nc.vector.tensor_mul(
                        attn_t[pb : pb + 64, ho, :], stage[0:DH, :], rep[0:64, :]
                    )

            if tcix + 1 < n_chunks:
                fcq.append((ts0, attn_t))
            else:
                for u in fc_units_end(ts0, attn_t):
                    u()

    nc.compile()
    return nc


def _prep_core_inputs(x, w_qkv, b_qkv, w_fc, b_fc, core):
    b, g = core // 4, core % 4
    rq = slice(256 * g, 256 * g + 256)
    rk = slice(1024 + 256 * g, 1024 + 256 * g + 256)
    rv = slice(2048 + 256 * g, 2048 + 256 * g + 256)
    wcat = np.concatenate([w_qkv[rq], w_qkv[rk], w_qkv[rv]], axis=0)  # [768, 1024]
    bq, bk, bv = b_qkv[rq], b_qkv[rk], b_qkv[rv]
    import ml_dtypes

    bf16 = ml_dtypes.bfloat16
    return {
        "xT": np.ascontiguousarray(x[b].T).astype(bf16),
        "wqkvT": np.ascontiguousarray(wcat.T).astype(bf16),
        "bqk": np.ascontiguousarray(
            np.stack([bq[0:128], bq[128:256], bk[0:128], bk[128:256]], axis=1)
        ),
        "bv_rep": np.ascontiguousarray(np.broadcast_to(bv, (128, 256))),
        "wfcT": np.ascontiguousarray(w_fc[:, 256 * g : 256 * g + 256].T).astype(bf16),
        "mask": np.triu(np.ones((128, 128), dtype=np.float32)).astype(bf16),
    }


def kernel(x, w_qkv, b_qkv, w_fc, b_fc):
    global LAST_RESULT
    x = np.asarray(x, dtype=np.float32)
    w_qkv = np.asarray(w_qkv, dtype=np.float32)
    b_qkv = np.asarray(b_qkv, dtype=np.float32)
    w_fc = np.asarray(w_fc, dtype=np.float32)
    b_fc = np.asarray(b_fc, dtype=np.float32)

    if "nc" not in _BUILD_CACHE:
        _BUILD_CACHE["nc"] = build()
    nc = _BUILD_CACHE["nc"]

    in_maps = [
        _prep_core_inputs(x, w_qkv, b_qkv, w_fc, b_fc, core) for core in range(8)
    ]
    res = run_bass_kernel_spmd(
        nc,
        in_maps,
        core_ids=list(range(8)),
        trace=bool(os.environ.get("MHA_TRACE")),
    )
    LAST_RESULT = res

    out = np.empty((B, T, C), dtype=np.float32)
    for b in range(B):
        acc = res.results[4 * b]["y"].astype(np.float32)
        for g in range(1, 4):
            acc = acc + res.results[4 * b + g]["y"].astype(np.float32)
        out[b] = acc + b_fc
    return out
